# revision 32
# baseline (speedup 1.0000x reference)
"""Trainium2 Bass kernel for a GPT-style transformer block.

Reference computation (per batch element):
    h  = LN1(x);  qkv = h @ qkv_w + qkv_b
    att = causal_softmax(q @ k.T / sqrt(64));  o = att @ v
    x  = x + o @ out_w + out_b
    h  = LN2(x);  u = relu(h @ fc_w + fc_b)
    y  = x + u @ proj_w + proj_b

Shapes: x [16, 1024, 256], 4 heads x 64, MLP hidden 1024.

Strategy: pure data-parallel over batch, 2 batch elements per core on 8
cores, no collectives.  Within a core:
  - All PE operands are bf16 (fp32 PSUM accumulation).  LN gamma/beta are
    folded into the following matmul weights on the host.
  - Seq-major <-> feature-major layout changes run on the DMA engines'
    XBAR transpose (SBUF->SBUF, 2-byte dtype), freeing the PE and ACT
    engines entirely; transposed tensors use the XBAR's native
    [dim_lo(128), t, et, q(128)] layout.
  - Attention scores are computed transposed (scoresT[k, q]); the causal
    mask is applied by accumulating a -1e9 strict-triangular matrix onto
    the diagonal PSUM block with one extra matmul (exp underflows to 0).
  - att@V runs in [q, dims] orientation (expT chunks are the stationary
    operand), so softmax denominators (from an all-ones column appended
    to V) land on the same partition as their row: normalization is a
    per-partition scalar multiply; reciprocals via the fast approximate
    DVE op.  The normalized o is XBAR-transposed back to feature-major
    for out_proj.
  - Engine balance: exp/relu/copies on ACT, stats/evac/recip/residuals
    on DVE, LN apply on GpSimd, masks folded into PE matmuls.
  - Input x and LN/transpose flow in half-batches of 4 seq tiles so the
    first qkv matmuls start as early as possible; a short run of dummy
    matmuls at kernel start keeps the PE busy while DMAs land, so the
    HAM clock gate is open when real work arrives.
"""

import sys

sys.path.insert(0, "/opt/trn_rl_repo")

import numpy as np

import concourse.bass as bass
import concourse.bacc as bacc
import concourse.tile as tile
from concourse import mybir
from concourse.masks import make_identity, make_upper_triangular

# Restrict the activation-table chooser to the one set that contains every
# function this kernel uses (exp, ln, copy/identity, relu) so the ACT
# engine never thrashes table loads.
_orig_get_act_tables = bacc.get_activation_tables


def _one_set_tables(module_arch):
    tabs = _orig_get_act_tables(module_arch)
    return {name: (fns if name == "natural_log_exp_and_others" else set())
            for name, fns in tabs.items()}


bacc.get_activation_tables = _one_set_tables

F32 = mybir.dt.float32
F32R = mybir.dt.float32r
BF16 = mybir.dt.bfloat16
AF = mybir.ActivationFunctionType
ALU = mybir.AluOpType

import os as _os

MM_DTYPE = _os.environ.get("BASS_MM_DTYPE", "bf16")
WARMUP_MM = int(_os.environ.get("BASS_WARMUP_MM", "36"))
# dummy matmuls per attention j-step that keep the PE activity monitor from
# re-throttling the clock during the ACT-bound exp stretches
FILLER = int(_os.environ.get("BASS_FILLER", "4"))


def _mmdt():
    return F32R if MM_DTYPE == "f32r" else BF16

NCORES = 8
B = 16
BPC = B // NCORES  # 2 batch elements per core
S = 1024
E = 256
H = 4
D = 64
FF = 1024
ST = S // 128  # 8 seq tiles
ET = E // 128  # 2 feature tiles
FT = FF // 128  # 8 mlp-hidden tiles
EPS = 1e-5

# ragged offsets for the causal expT store: tile j holds q in [128j, S)
EOFF = [0]
for _j in range(ST):
    EOFF.append(EOFF[-1] + (S - 128 * _j))
ETOT = EOFF[ST]  # 4608


def _r(ap):
    """View an fp32 AP as float32r (no-op for bf16/f32r tiles)."""
    if ap.dtype in (F32R, BF16):
        return ap
    return ap.bitcast(F32R)


def _bcast(ap_1d, parts):
    """Broadcast a 1-partition AP across `parts` partitions (step-0 AP)."""
    return bass.AP(tensor=ap_1d.tensor, offset=ap_1d.offset,
                   ap=[[0, parts]] + list(ap_1d.ap))


def build_bass(reps=1):
    MMDT = _mmdt()
    nc = bacc.Bacc(None, target_bir_lowering=False, debug=False)

    # ---- DRAM I/O ----
    x_in = nc.dram_tensor("x", [BPC, S, E], F32, kind="ExternalInput")
    qk_w = nc.dram_tensor("qk_w", [E, 512], MMDT, kind="ExternalInput")
    qk_bc = nc.dram_tensor("qk_bc", [128, 4], F32, kind="ExternalInput")
    wv = nc.dram_tensor("wv", [E, H * (D + 1)], MMDT, kind="ExternalInput")
    bv_row = nc.dram_tensor("bv_row", [1, H * (D + 1)], MMDT, kind="ExternalInput")
    out_w = nc.dram_tensor("out_w", [E, E], MMDT, kind="ExternalInput")
    outb_row = nc.dram_tensor("outb_row", [1, E], MMDT, kind="ExternalInput")
    fc_w = nc.dram_tensor("fc_w", [E, FF], MMDT, kind="ExternalInput")
    fc_bt = nc.dram_tensor("fc_bt", [128, FT], F32, kind="ExternalInput")
    proj_w = nc.dram_tensor("proj_w", [FF, E], MMDT, kind="ExternalInput")
    projb_row = nc.dram_tensor("projb_row", [1, E], MMDT, kind="ExternalInput")
    y_out = nc.dram_tensor("y", [BPC, S, E], F32, kind="ExternalOutput")
    # scratch for the softmax-denominator reshape round trip
    sums_hbm = nc.dram_tensor("sums_hbm", [BPC, H, S], F32, kind="Internal")
    rec_hbm = nc.dram_tensor("rec_hbm", [BPC, H, S], F32, kind="Internal")

    VA = H * (D + 1)  # 260

    with tile.TileContext(nc) as tc:
        wp = tc.alloc_tile_pool(name="weights", bufs=1)
        sp = tc.alloc_tile_pool(name="small", bufs=2)
        bp2 = tc.alloc_tile_pool(name="big2", bufs=2)
        bp1 = tc.alloc_tile_pool(name="big1", bufs=1)
        ep = tc.alloc_tile_pool(name="expt", bufs=1)
        op = tc.alloc_tile_pool(name="oscp", bufs=4)
        psA = tc.alloc_tile_pool(name="psA", bufs=2, space="PSUM")
        psB = tc.alloc_tile_pool(name="psB", bufs=2, space="PSUM")
        psO = tc.alloc_tile_pool(name="psO", bufs=2, space="PSUM")

        # ---- constants first: the gpsimd queue must build the masks before
        # anything else so the first transposes/scores aren't blocked ----
        eps_col = wp.tile([128, 1], F32)
        nc.vector.memset(eps_col, EPS)
        ones_row = wp.tile([1, S], MMDT)
        nc.vector.memset(ones_row, 1.0)
        ident = wp.tile([128, 128], MMDT)
        make_identity(nc, ident)
        # additive causal mask for the scoresT diagonal block:
        # matmul(negtri.T @ ident) adds -1e9 where q < k
        negtri = wp.tile([128, 128], MMDT)
        make_upper_triangular(nc, negtri, val=-1e9, diag=False)

        # ---- PE warmup: keep the HAM clock gate open during input DMA ----
        warm = wp.tile([128, 512], MMDT)
        nc.vector.memset(warm, 0.25)
        if WARMUP_MM:
            for _ in range(WARMUP_MM):
                wps = psO.tile([128, 512], F32, tag="psO")
                nc.tensor.matmul(wps, _r(warm[:, 0:128]), _r(warm),
                                 start=True, stop=True)

        # ---- weights needed early (attention input projections) ----
        qk_w_sb = wp.tile([128, ET, 512], MMDT)
        nc.gpsimd.dma_start(out=qk_w_sb, in_=qk_w[:, :].rearrange("(t p) c -> p t c", p=128))
        qk_bc_sb = wp.tile([128, 4], F32)
        nc.gpsimd.dma_start(out=qk_bc_sb, in_=qk_bc[:, :])
        wv_sb = wp.tile([128, ET, VA], MMDT)
        nc.gpsimd.dma_start(out=wv_sb, in_=wv[:, :].rearrange("(t p) c -> p t c", p=128))
        bv_sb = wp.tile([1, VA], MMDT)
        nc.gpsimd.dma_start(out=bv_sb, in_=bv_row[:, :])

        # late weights: tiles allocated now, DMAs emitted after p1(0) so the
        # gpsimd queue reaches the first LN applies quickly
        out_w_sb = wp.tile([128, ET, E], MMDT)
        outb_sb = wp.tile([1, E], MMDT)
        fc_w_sb = wp.tile([128, ET, FF], MMDT)
        fc_bt_sb = wp.tile([128, FT], F32)
        proj_w_sb = wp.tile([128, FT, E], MMDT)
        projb_sb = wp.tile([1, E], MMDT)

        def emit_late_weights():
            nc.gpsimd.dma_start(out=out_w_sb, in_=out_w[:, :].rearrange("(t p) c -> p t c", p=128))
            nc.gpsimd.dma_start(out=outb_sb, in_=outb_row[:, :])
            nc.gpsimd.dma_start(out=fc_w_sb, in_=fc_w[:, :].rearrange("(t p) c -> p t c", p=128))
            nc.gpsimd.dma_start(out=fc_bt_sb, in_=fc_bt[:, :])
            nc.gpsimd.dma_start(out=proj_w_sb, in_=proj_w[:, :].rearrange("(t p) c -> p t c", p=128))
            nc.gpsimd.dma_start(out=projb_sb, in_=projb_row[:, :])

        def emit_ln_half(src, dst_fn, half, rstd, nmr):
            """LayerNorm of seq tiles [4*half, 4*half+4): stats on DVE, rstd
            on ACT, apply on GpSimd -> bf16 dst."""
            t0 = 4 * half
            stats = sp.tile([128, 4, 6], F32, tag="bnstats")
            mv = sp.tile([128, 4, 2], F32, tag="bnaggr")
            for i in range(4):
                nc.vector.bn_stats(out=stats[:, i, :], in_=src[:, t0 + i, :])
                nc.vector.bn_aggr(out=mv[:, i, :], in_=stats[:, i, :])
            sl = slice(t0, t0 + 4)
            # rstd = exp(-0.5 * ln(var + eps))
            nc.scalar.activation(rstd[:, sl], mv[:, :, 1], AF.Ln, bias=eps_col)
            nc.scalar.activation(rstd[:, sl], rstd[:, sl], AF.Exp, scale=-0.5)
            nc.vector.tensor_mul(nmr[:, sl], mv[:, :, 0], rstd[:, sl])
            nc.vector.tensor_scalar_mul(nmr[:, sl], nmr[:, sl], -1.0)
            for i in range(4):
                t = t0 + i
                nc.gpsimd.tensor_scalar(
                    out=dst_fn(t), in0=src[:, t, :],
                    scalar1=rstd[:, t:t + 1], scalar2=nmr[:, t:t + 1],
                    op0=ALU.mult, op1=ALU.add)

        def emit_transpose_half(h_sb, hT, half):
            """XBAR transpose seq tiles [4h, 4h+4) of h_sb [128, ST, E] into
            hT [128, ST, ET, 128] (layout [e_lo, t, et, q])."""
            t0 = 4 * half
            nc.sync.dma_start_transpose(
                out=hT[:, t0:t0 + 4, :, :], in_=h_sb[:, t0:t0 + 4, :])

        def emit_p1(b, xs):
            # ---- LN1 -> h1 (bf16) -> h1T via XBAR, in halves ----
            h1 = bp2.tile([128, ST, E], MMDT, tag="h1")
            h1T = bp2.tile([128, ST, ET, 128], MMDT, tag="hT")
            rstd = sp.tile([128, ST], F32, tag="rstd")
            nmr = sp.tile([128, ST], F32, tag="nmr")
            for half in (0, 1):
                emit_ln_half(xs, lambda t: h1[:, t, :], half, rstd, nmr)
                emit_transpose_half(h1, h1T, half)

            # ---- qkv ----
            # q/k feature-major: qkT[:, m, :]; m=0: q heads 0-1, m=1: q heads
            # 2-3, m=2: k heads 0-1, m=3: k heads 2-3 (rows = 2x64 head dims)
            qkT = bp2.tile([128, 4, S], MMDT, tag="qkT")
            for m in range(4):
                ps = psB.tile([128, S], F32, tag="psB")
                for c in range(2):
                    sl = slice(512 * c, 512 * (c + 1))
                    nc.tensor.matmul(ps[:, sl], _r(qk_w_sb[:, 0, 128 * m:128 * (m + 1)]),
                                     _r(h1T[:, 4 * c:4 * c + 4, 0, :]),
                                     start=True, stop=False)
                    nc.tensor.matmul(ps[:, sl], _r(qk_w_sb[:, 1, 128 * m:128 * (m + 1)]),
                                     _r(h1T[:, 4 * c:4 * c + 4, 1, :]),
                                     start=False, stop=True)
                    # evacuate + per-partition bias add per chunk
                    nc.vector.tensor_scalar(
                        out=qkT[:, m, sl], in0=ps[:, sl],
                        scalar1=qk_bc_sb[:, m:m + 1],
                        scalar2=None, op0=ALU.add)

            # v seq-major with per-head all-ones column: vv[:, t, 65h+64] == 1
            vv = bp2.tile([128, ST, VA], MMDT, tag="vv")
            for t in range(ST):
                ps = psA.tile([128, VA], F32, tag="psA")
                nc.tensor.matmul(ps, _r(h1T[:, t, 0, :]),
                                 _r(wv_sb[:, 0, :]), start=True, stop=False)
                nc.tensor.matmul(ps, _r(h1T[:, t, 1, :]),
                                 _r(wv_sb[:, 1, :]), start=False, stop=False)
                # bias row (rank-1: ones_col @ bv_row); also writes the 1.0 col
                nc.tensor.matmul(ps, _r(ones_row[0:1, 128 * t:128 * (t + 1)]),
                                 _r(bv_sb[0:1, :]), start=False, stop=True)
                nc.scalar.copy(vv[:, t, :], ps)

            return qkT, vv

        def emit_p2(b, qkT, vv):
            # ---- attention: scores j-loop with att@V zippered in ----
            expTs = {h: ep.tile([128, ETOT], MMDT, tag=f"expT{h}",
                                name=f"expT{h}b{b}") for h in range(H)}
            o_sb = bp2.tile([128, ST, E], MMDT, tag="osb")

            def emit_attv(t):
                po = psO.tile([128, VA], F32, tag="psO")
                for h in range(H):
                    expT = expTs[h]
                    for j in range(t + 1):
                        a = EOFF[j] + 128 * (t - j)
                        nc.tensor.matmul(
                            po[:, 65 * h:65 * h + 65],
                            _r(expT[:, a:a + 128]),
                            _r(vv[:, j, 65 * h:65 * h + 65]),
                            start=(j == 0), stop=(j == t))
                pr = po.rearrange("p (h x) -> p h x", x=65)
                sums = sp.tile([128, H, 1], F32, tag="sums")
                nc.vector.tensor_copy(sums, pr[:, :, 64:65])
                rec4 = sp.tile([128, H, 1], F32, tag="rec4")
                nc.vector.reciprocal_approx_fast(rec4, sums)
                for h in range(H):
                    nc.vector.tensor_scalar(
                        out=o_sb[:, t, 64 * h:64 * h + 64], in0=pr[:, h, 0:64],
                        scalar1=rec4[:, h, :], scalar2=None, op0=ALU.mult)
                if FILLER:
                    fps = psA.tile([128, 256], F32, tag="psA")
                    nc.tensor.matmul(fps, _r(warm[:, 0:128]), _r(warm[:, 0:256]),
                                     start=True, stop=True)

            for j in range(ST):
                w0 = 128 * j
                pss = {}
                for pair in range(2):
                    for hh in range(2):
                        h = 2 * pair + hh
                        qsl = slice(64 * hh, 64 * hh + 64)
                        ps = psB.tile([128, S], F32, tag="psB", name=f"psSC{h}")
                        pss[h] = ps
                        for c in range(w0 // 512, 2):
                            a = max(w0, 512 * c)
                            nc.tensor.matmul(
                                ps[:, a:512 * (c + 1)],
                                _r(qkT[qsl, 2 + pair, w0:w0 + 128]),
                                _r(qkT[qsl, pair, a:512 * (c + 1)]),
                                start=True, stop=True)
                    for hh in range(2):
                        nc.tensor.matmul(
                            pss[2 * pair + hh][:, w0:w0 + 128], _r(negtri),
                            _r(ident), start=False, stop=True)
                    for hh in range(2):
                        h = 2 * pair + hh
                        nc.scalar.activation(
                            expTs[h][:, EOFF[j]:EOFF[j] + (S - w0)],
                            pss[h][:, w0:S], AF.Exp, scale=0.125)
                if j >= 2:
                    emit_attv(j - 2)
                if FILLER:
                    for _ in range(FILLER):
                        fps = psA.tile([128, 256], F32, tag="psA")
                        nc.tensor.matmul(fps, _r(warm[:, 0:128]),
                                         _r(warm[:, 0:256]),
                                         start=True, stop=True)
            emit_attv(ST - 2)
            emit_attv(ST - 1)
            oT = bp2.tile([128, ST, ET, 128], MMDT, tag="oT")
            for half in (0, 1):
                emit_transpose_half(o_sb, oT, half)
            return oT

        def emit_p3(b, xs, oT):
            # ---- out_proj + residual -> x2 ----
            x2 = bp1.tile([128, ST, E], F32, tag="x2")
            for t in range(ST):
                ps = psA.tile([128, E], F32, tag="psA")
                nc.tensor.matmul(ps, _r(oT[:, t, 0, :]),
                                 _r(out_w_sb[:, 0, :]), start=True, stop=False)
                nc.tensor.matmul(ps, _r(oT[:, t, 1, :]),
                                 _r(out_w_sb[:, 1, :]), start=False, stop=False)
                nc.tensor.matmul(ps, _r(ones_row[0:1, 128 * t:128 * (t + 1)]),
                                 _r(outb_sb[0:1, :]), start=False, stop=True)
                nc.vector.tensor_add(x2[:, t, :], ps, xs[:, t, :])
                if FILLER:
                    fps = psO.tile([128, 512], F32, tag="psO")
                    nc.tensor.matmul(fps, _r(warm[:, 0:128]), _r(warm),
                                     start=True, stop=True)

            # ---- LN2 -> h2 (bf16) -> h2T via XBAR ----
            h2 = bp2.tile([128, ST, E], MMDT, tag="h1")
            h2T = bp2.tile([128, ST, ET, 128], MMDT, tag="hT")
            rstd = sp.tile([128, ST], F32, tag="rstd")
            nmr = sp.tile([128, ST], F32, tag="nmr")
            for half in (0, 1):
                emit_ln_half(x2, lambda t: h2[:, t, :], half, rstd, nmr)
                emit_transpose_half(h2, h2T, half)

            # ---- MLP fc + relu (uT feature-major) ----
            uT = bp1.tile([128, FT, S], MMDT, tag="uT")
            for m in range(FT):
                ps = psB.tile([128, S], F32, tag="psB")
                for c in range(2):
                    sl = slice(512 * c, 512 * (c + 1))
                    nc.tensor.matmul(ps[:, sl], _r(fc_w_sb[:, 0, 128 * m:128 * (m + 1)]),
                                     _r(h2T[:, 4 * c:4 * c + 4, 0, :]),
                                     start=True, stop=False)
                    nc.tensor.matmul(ps[:, sl], _r(fc_w_sb[:, 1, 128 * m:128 * (m + 1)]),
                                     _r(h2T[:, 4 * c:4 * c + 4, 1, :]),
                                     start=False, stop=True)
                nc.scalar.activation(uT[:, m, :], ps, AF.Relu,
                                     bias=fc_bt_sb[:, m:m + 1])
                if FILLER:
                    fps = psO.tile([128, 512], F32, tag="psO")
                    nc.tensor.matmul(fps, _r(warm[:, 0:128]), _r(warm),
                                     start=True, stop=True)

            # ---- proj + residual -> y (reuses the xs tile in place) ----
            for t in range(ST):
                ps = psA.tile([128, E], F32, tag="psA")
                for m in range(FT):
                    nc.tensor.matmul(ps, _r(uT[:, m, 128 * t:128 * (t + 1)]),
                                     _r(proj_w_sb[:, m, :]),
                                     start=(m == 0), stop=False)
                nc.tensor.matmul(ps, _r(ones_row[0:1, 128 * t:128 * (t + 1)]),
                                 _r(projb_sb[0:1, :]), start=False, stop=True)
                nc.vector.tensor_add(xs[:, t, :], ps, x2[:, t, :])
                if FILLER:
                    fps = psO.tile([128, 512], F32, tag="psO")
                    nc.tensor.matmul(fps, _r(warm[:, 0:128]), _r(warm),
                                     start=True, stop=True)
            for half in (0, 1):
                t0 = 4 * half
                nc.sync.dma_start(
                    out=y_out[b, 128 * t0:128 * (t0 + 4), :].rearrange(
                        "(t p) e -> p t e", p=128),
                    in_=xs[:, t0:t0 + 4, :])

        for _rep in range(reps):
            xs_tiles = []
            for b in range(BPC):
                xs = bp2.tile([128, ST, E], F32, tag="xs", name=f"xs{b}")
                for half in (0, 1):
                    t0 = 4 * half
                    nc.sync.dma_start(
                        out=xs[:, t0:t0 + 4, :],
                        in_=x_in[b, 128 * t0:128 * (t0 + 4), :].rearrange(
                            "(t p) e -> p t e", p=128))
                xs_tiles.append(xs)
            # strict phase order so every engine queue's FIFO matches true
            # dependency order (an out-of-order emission head-blocks the
            # in-order engine queues on cross-batch waits)
            qv0 = emit_p1(0, xs_tiles[0])
            emit_late_weights()
            qv1 = emit_p1(1, xs_tiles[1])
            oT0 = emit_p2(0, *qv0)
            oT1 = emit_p2(1, *qv1)
            emit_p3(0, xs_tiles[0], oT0)
            emit_p3(1, xs_tiles[1], oT1)

        for p in (psO, psB, psA, op, ep, bp1, bp2, sp, wp):
            p.release()

    nc.compile()
    return nc


def host_prep(inputs):
    """Fold LN params into weights; build the DRAM-side weight layouts."""
    f = np.float32
    qkv_w = np.asarray(inputs["qkv_w"], f)
    qkv_b = np.asarray(inputs["qkv_b"], f)
    ln1_g = np.asarray(inputs["ln1_g"], f)
    ln1_b = np.asarray(inputs["ln1_b"], f)
    ln2_g = np.asarray(inputs["ln2_g"], f)
    ln2_b = np.asarray(inputs["ln2_b"], f)
    fc_w = np.asarray(inputs["fc_w"], f)
    fc_b = np.asarray(inputs["fc_b"], f)

    W1 = qkv_w * ln1_g[:, None]
    b1 = qkv_b + ln1_b @ qkv_w
    W2 = fc_w * ln2_g[:, None]
    b2 = fc_b + ln2_b @ fc_w

    qk_w = np.ascontiguousarray(W1[:, :512])
    qk_bc = np.ascontiguousarray(b1[:512].reshape(4, 128).T)
    wv = np.zeros((E, H * (D + 1)), f)
    bv = np.zeros((1, H * (D + 1)), f)
    for h in range(H):
        wv[:, 65 * h:65 * h + 64] = W1[:, 512 + 64 * h:512 + 64 * (h + 1)]
        bv[0, 65 * h:65 * h + 64] = b1[512 + 64 * h:512 + 64 * (h + 1)]
        bv[0, 65 * h + 64] = 1.0
    fc_bt = np.ascontiguousarray(b2.reshape(FT, 128).T)

    import ml_dtypes
    wdt = np.float32 if MM_DTYPE == "f32r" else ml_dtypes.bfloat16

    return {
        "qk_w": qk_w.astype(wdt),
        "qk_bc": qk_bc,
        "wv": wv.astype(wdt),
        "bv_row": bv.astype(wdt),
        "out_w": np.asarray(inputs["out_w"], f).astype(wdt),
        "outb_row": np.asarray(inputs["out_b"], f).reshape(1, E).astype(wdt),
        "fc_w": W2.astype(wdt),
        "fc_bt": fc_bt,
        "proj_w": np.asarray(inputs["proj_w"], f).astype(wdt),
        "projb_row": np.asarray(inputs["proj_b"], f).reshape(1, E).astype(wdt),
    }


_NC_CACHE = None


def _get_nc():
    global _NC_CACHE
    if _NC_CACHE is None:
        _NC_CACHE = build_bass()
    return _NC_CACHE


def run(inputs, trace=False):
    from concourse.bass_utils import run_bass_kernel_spmd

    nc = _get_nc()
    weights = host_prep(inputs)
    x = np.asarray(inputs["x"], np.float32)
    in_maps = []
    for c in range(NCORES):
        m = dict(weights)
        m["x"] = np.ascontiguousarray(x[BPC * c:BPC * (c + 1)])
        in_maps.append(m)
    res = run_bass_kernel_spmd(nc, in_maps, core_ids=list(range(NCORES)),
                               trace=trace)
    y = np.concatenate([res.results[c]["y"] for c in range(NCORES)], axis=0)
    return y, res


def kernel(**inputs):
    y, _ = run(inputs)
    return y


# revision 33
# speedup vs baseline: 1.0086x; 1.0086x over previous
"""Trainium2 Bass kernel for a GPT-style transformer block.

Reference computation (per batch element):
    h  = LN1(x);  qkv = h @ qkv_w + qkv_b
    att = causal_softmax(q @ k.T / sqrt(64));  o = att @ v
    x  = x + o @ out_w + out_b
    h  = LN2(x);  u = relu(h @ fc_w + fc_b)
    y  = x + u @ proj_w + proj_b

Shapes: x [16, 1024, 256], 4 heads x 64, MLP hidden 1024.

Strategy: pure data-parallel over batch, 2 batch elements per core on 8
cores, no collectives.  Within a core:
  - All PE operands are bf16 (fp32 PSUM accumulation).  LN gamma/beta are
    folded into the following matmul weights on the host.
  - Seq-major <-> feature-major layout changes run on the DMA engines'
    XBAR transpose (SBUF->SBUF, 2-byte dtype), freeing the PE and ACT
    engines entirely; transposed tensors use the XBAR's native
    [dim_lo(128), t, et, q(128)] layout.
  - Attention scores are computed transposed (scoresT[k, q]); the causal
    mask is applied by accumulating a -1e9 strict-triangular matrix onto
    the diagonal PSUM block with one extra matmul (exp underflows to 0).
  - att@V runs in [q, dims] orientation (expT chunks are the stationary
    operand), so softmax denominators (from an all-ones column appended
    to V) land on the same partition as their row: normalization is a
    per-partition scalar multiply; reciprocals via the fast approximate
    DVE op.  The normalized o is XBAR-transposed back to feature-major
    for out_proj.
  - Engine balance: exp/relu/copies on ACT, stats/evac/recip/residuals
    on DVE, LN apply on GpSimd, masks folded into PE matmuls.
  - Input x and LN/transpose flow in half-batches of 4 seq tiles so the
    first qkv matmuls start as early as possible; a short run of dummy
    matmuls at kernel start keeps the PE busy while DMAs land, so the
    HAM clock gate is open when real work arrives.
"""

import sys

sys.path.insert(0, "/opt/trn_rl_repo")

import numpy as np

import concourse.bass as bass
import concourse.bacc as bacc
import concourse.tile as tile
from concourse import mybir
from concourse.masks import make_identity, make_upper_triangular

# Restrict the activation-table chooser to the one set that contains every
# function this kernel uses (exp, ln, copy/identity, relu) so the ACT
# engine never thrashes table loads.
_orig_get_act_tables = bacc.get_activation_tables


def _one_set_tables(module_arch):
    tabs = _orig_get_act_tables(module_arch)
    return {name: (fns if name == "natural_log_exp_and_others" else set())
            for name, fns in tabs.items()}


bacc.get_activation_tables = _one_set_tables

F32 = mybir.dt.float32
F32R = mybir.dt.float32r
BF16 = mybir.dt.bfloat16
AF = mybir.ActivationFunctionType
ALU = mybir.AluOpType

import os as _os

MM_DTYPE = _os.environ.get("BASS_MM_DTYPE", "bf16")
WARMUP_MM = int(_os.environ.get("BASS_WARMUP_MM", "28"))
# dummy matmuls per attention j-step that keep the PE activity monitor from
# re-throttling the clock during the ACT-bound exp stretches
FILLER = int(_os.environ.get("BASS_FILLER", "4"))


def _mmdt():
    return F32R if MM_DTYPE == "f32r" else BF16

NCORES = 8
B = 16
BPC = B // NCORES  # 2 batch elements per core
S = 1024
E = 256
H = 4
D = 64
FF = 1024
ST = S // 128  # 8 seq tiles
ET = E // 128  # 2 feature tiles
FT = FF // 128  # 8 mlp-hidden tiles
EPS = 1e-5

# ragged offsets for the causal expT store: tile j holds q in [128j, S)
EOFF = [0]
for _j in range(ST):
    EOFF.append(EOFF[-1] + (S - 128 * _j))
ETOT = EOFF[ST]  # 4608


def _r(ap):
    """View an fp32 AP as float32r (no-op for bf16/f32r tiles)."""
    if ap.dtype in (F32R, BF16):
        return ap
    return ap.bitcast(F32R)


def _bcast(ap_1d, parts):
    """Broadcast a 1-partition AP across `parts` partitions (step-0 AP)."""
    return bass.AP(tensor=ap_1d.tensor, offset=ap_1d.offset,
                   ap=[[0, parts]] + list(ap_1d.ap))


def build_bass(reps=1):
    MMDT = _mmdt()
    nc = bacc.Bacc(None, target_bir_lowering=False, debug=False)

    # ---- DRAM I/O ----
    x_in = nc.dram_tensor("x", [BPC, S, E], F32, kind="ExternalInput")
    qk_w = nc.dram_tensor("qk_w", [E, 512], MMDT, kind="ExternalInput")
    qk_bc = nc.dram_tensor("qk_bc", [128, 4], F32, kind="ExternalInput")
    wv = nc.dram_tensor("wv", [E, H * (D + 1)], MMDT, kind="ExternalInput")
    bv_row = nc.dram_tensor("bv_row", [1, H * (D + 1)], MMDT, kind="ExternalInput")
    out_w = nc.dram_tensor("out_w", [E, E], MMDT, kind="ExternalInput")
    outb_row = nc.dram_tensor("outb_row", [1, E], MMDT, kind="ExternalInput")
    fc_w = nc.dram_tensor("fc_w", [E, FF], MMDT, kind="ExternalInput")
    fc_bt = nc.dram_tensor("fc_bt", [128, FT], F32, kind="ExternalInput")
    proj_w = nc.dram_tensor("proj_w", [FF, E], MMDT, kind="ExternalInput")
    projb_row = nc.dram_tensor("projb_row", [1, E], MMDT, kind="ExternalInput")
    y_out = nc.dram_tensor("y", [BPC, S, E], F32, kind="ExternalOutput")
    # scratch for the softmax-denominator reshape round trip
    sums_hbm = nc.dram_tensor("sums_hbm", [BPC, H, S], F32, kind="Internal")
    rec_hbm = nc.dram_tensor("rec_hbm", [BPC, H, S], F32, kind="Internal")

    VA = H * (D + 1)  # 260

    with tile.TileContext(nc) as tc:
        wp = tc.alloc_tile_pool(name="weights", bufs=1)
        sp = tc.alloc_tile_pool(name="small", bufs=2)
        bp2 = tc.alloc_tile_pool(name="big2", bufs=2)
        bp1 = tc.alloc_tile_pool(name="big1", bufs=1)
        ep = tc.alloc_tile_pool(name="expt", bufs=1)
        op = tc.alloc_tile_pool(name="oscp", bufs=4)
        psA = tc.alloc_tile_pool(name="psA", bufs=2, space="PSUM")
        psB = tc.alloc_tile_pool(name="psB", bufs=2, space="PSUM")
        psO = tc.alloc_tile_pool(name="psO", bufs=2, space="PSUM")

        # ---- constants first: the gpsimd queue must build the masks before
        # anything else so the first transposes/scores aren't blocked ----
        eps_col = wp.tile([128, 1], F32)
        nc.vector.memset(eps_col, EPS)
        ones_row = wp.tile([1, S], MMDT)
        nc.vector.memset(ones_row, 1.0)
        ident = wp.tile([128, 128], MMDT)
        make_identity(nc, ident)
        # additive causal mask for the scoresT diagonal block:
        # matmul(negtri.T @ ident) adds -1e9 where q < k
        negtri = wp.tile([128, 128], MMDT)
        make_upper_triangular(nc, negtri, val=-1e9, diag=False)

        # ---- PE warmup: keep the HAM clock gate open during input DMA ----
        warm = wp.tile([128, 512], MMDT)
        nc.vector.memset(warm, 0.25)
        if WARMUP_MM:
            for _ in range(WARMUP_MM):
                wps = psO.tile([128, 512], F32, tag="psO")
                nc.tensor.matmul(wps, _r(warm[:, 0:128]), _r(warm),
                                 start=True, stop=True)

        # ---- weights needed early (attention input projections) ----
        qk_w_sb = wp.tile([128, ET, 512], MMDT)
        nc.gpsimd.dma_start(out=qk_w_sb, in_=qk_w[:, :].rearrange("(t p) c -> p t c", p=128))
        qk_bc_sb = wp.tile([128, 4], F32)
        nc.gpsimd.dma_start(out=qk_bc_sb, in_=qk_bc[:, :])
        wv_sb = wp.tile([128, ET, VA], MMDT)
        nc.gpsimd.dma_start(out=wv_sb, in_=wv[:, :].rearrange("(t p) c -> p t c", p=128))
        bv_sb = wp.tile([1, VA], MMDT)
        nc.gpsimd.dma_start(out=bv_sb, in_=bv_row[:, :])

        # late weights: tiles allocated now, DMAs emitted after p1(0) so the
        # gpsimd queue reaches the first LN applies quickly
        out_w_sb = wp.tile([128, ET, E], MMDT)
        outb_sb = wp.tile([1, E], MMDT)
        fc_w_sb = wp.tile([128, ET, FF], MMDT)
        fc_bt_sb = wp.tile([128, FT], F32)
        proj_w_sb = wp.tile([128, FT, E], MMDT)
        projb_sb = wp.tile([1, E], MMDT)

        def emit_late_weights():
            nc.gpsimd.dma_start(out=out_w_sb, in_=out_w[:, :].rearrange("(t p) c -> p t c", p=128))
            nc.gpsimd.dma_start(out=outb_sb, in_=outb_row[:, :])
            nc.gpsimd.dma_start(out=fc_w_sb, in_=fc_w[:, :].rearrange("(t p) c -> p t c", p=128))
            nc.gpsimd.dma_start(out=fc_bt_sb, in_=fc_bt[:, :])
            nc.gpsimd.dma_start(out=proj_w_sb, in_=proj_w[:, :].rearrange("(t p) c -> p t c", p=128))
            nc.gpsimd.dma_start(out=projb_sb, in_=projb_row[:, :])

        def emit_ln_half(src, dst_fn, half, rstd, nmr):
            """LayerNorm of seq tiles [4*half, 4*half+4): stats on DVE, rstd
            on ACT, apply on GpSimd -> bf16 dst."""
            t0 = 4 * half
            stats = sp.tile([128, 4, 6], F32, tag="bnstats")
            mv = sp.tile([128, 4, 2], F32, tag="bnaggr")
            for i in range(4):
                nc.vector.bn_stats(out=stats[:, i, :], in_=src[:, t0 + i, :])
                nc.vector.bn_aggr(out=mv[:, i, :], in_=stats[:, i, :])
            sl = slice(t0, t0 + 4)
            # rstd = exp(-0.5 * ln(var + eps))
            nc.scalar.activation(rstd[:, sl], mv[:, :, 1], AF.Ln, bias=eps_col)
            nc.scalar.activation(rstd[:, sl], rstd[:, sl], AF.Exp, scale=-0.5)
            nc.vector.tensor_mul(nmr[:, sl], mv[:, :, 0], rstd[:, sl])
            nc.vector.tensor_scalar_mul(nmr[:, sl], nmr[:, sl], -1.0)
            for i in range(4):
                t = t0 + i
                nc.gpsimd.tensor_scalar(
                    out=dst_fn(t), in0=src[:, t, :],
                    scalar1=rstd[:, t:t + 1], scalar2=nmr[:, t:t + 1],
                    op0=ALU.mult, op1=ALU.add)

        def emit_transpose_half(h_sb, hT, half):
            """XBAR transpose seq tiles [4h, 4h+4) of h_sb [128, ST, E] into
            hT [128, ST, ET, 128] (layout [e_lo, t, et, q])."""
            t0 = 4 * half
            nc.sync.dma_start_transpose(
                out=hT[:, t0:t0 + 4, :, :], in_=h_sb[:, t0:t0 + 4, :])

        def emit_p1(b, xs):
            # ---- LN1 -> h1 (bf16) -> h1T via XBAR, in halves ----
            h1 = bp2.tile([128, ST, E], MMDT, tag="h1")
            h1T = bp2.tile([128, ST, ET, 128], MMDT, tag="hT")
            rstd = sp.tile([128, ST], F32, tag="rstd")
            nmr = sp.tile([128, ST], F32, tag="nmr")
            for half in (0, 1):
                emit_ln_half(xs, lambda t: h1[:, t, :], half, rstd, nmr)
                emit_transpose_half(h1, h1T, half)

            # ---- qkv ----
            # q/k feature-major: qkT[:, m, :]; m=0: q heads 0-1, m=1: q heads
            # 2-3, m=2: k heads 0-1, m=3: k heads 2-3 (rows = 2x64 head dims)
            qkT = bp2.tile([128, 4, S], MMDT, tag="qkT")
            for m in range(4):
                ps = psB.tile([128, S], F32, tag="psB")
                for c in range(2):
                    sl = slice(512 * c, 512 * (c + 1))
                    nc.tensor.matmul(ps[:, sl], _r(qk_w_sb[:, 0, 128 * m:128 * (m + 1)]),
                                     _r(h1T[:, 4 * c:4 * c + 4, 0, :]),
                                     start=True, stop=False)
                    nc.tensor.matmul(ps[:, sl], _r(qk_w_sb[:, 1, 128 * m:128 * (m + 1)]),
                                     _r(h1T[:, 4 * c:4 * c + 4, 1, :]),
                                     start=False, stop=True)
                    # evacuate + per-partition bias add per chunk
                    nc.vector.tensor_scalar(
                        out=qkT[:, m, sl], in0=ps[:, sl],
                        scalar1=qk_bc_sb[:, m:m + 1],
                        scalar2=None, op0=ALU.add)

            # v seq-major with per-head all-ones column: vv[:, t, 65h+64] == 1
            vv = bp2.tile([128, ST, VA], MMDT, tag="vv")
            for t in range(ST):
                ps = psA.tile([128, VA], F32, tag="psA")
                nc.tensor.matmul(ps, _r(h1T[:, t, 0, :]),
                                 _r(wv_sb[:, 0, :]), start=True, stop=False)
                nc.tensor.matmul(ps, _r(h1T[:, t, 1, :]),
                                 _r(wv_sb[:, 1, :]), start=False, stop=False)
                # bias row (rank-1: ones_col @ bv_row); also writes the 1.0 col
                nc.tensor.matmul(ps, _r(ones_row[0:1, 128 * t:128 * (t + 1)]),
                                 _r(bv_sb[0:1, :]), start=False, stop=True)
                nc.scalar.copy(vv[:, t, :], ps)

            return qkT, vv

        def emit_p2(b, qkT, vv):
            # ---- attention: scores j-loop with att@V zippered in ----
            expTs = {h: ep.tile([128, ETOT], MMDT, tag=f"expT{h}",
                                name=f"expT{h}b{b}") for h in range(H)}
            o_sb = bp2.tile([128, ST, E], MMDT, tag="osb")

            def emit_attv(t):
                po = psO.tile([128, VA], F32, tag="psO")
                for h in range(H):
                    expT = expTs[h]
                    for j in range(t + 1):
                        a = EOFF[j] + 128 * (t - j)
                        nc.tensor.matmul(
                            po[:, 65 * h:65 * h + 65],
                            _r(expT[:, a:a + 128]),
                            _r(vv[:, j, 65 * h:65 * h + 65]),
                            start=(j == 0), stop=(j == t))
                pr = po.rearrange("p (h x) -> p h x", x=65)
                sums = sp.tile([128, H, 1], F32, tag="sums")
                nc.vector.tensor_copy(sums, pr[:, :, 64:65])
                rec4 = sp.tile([128, H, 1], F32, tag="rec4")
                nc.vector.reciprocal_approx_fast(rec4, sums)
                for h in range(H):
                    nc.vector.tensor_scalar(
                        out=o_sb[:, t, 64 * h:64 * h + 64], in0=pr[:, h, 0:64],
                        scalar1=rec4[:, h, :], scalar2=None, op0=ALU.mult)

            for j in range(ST):
                w0 = 128 * j
                pss = {}
                for pair in range(2):
                    for hh in range(2):
                        h = 2 * pair + hh
                        qsl = slice(64 * hh, 64 * hh + 64)
                        ps = psB.tile([128, S], F32, tag="psB", name=f"psSC{h}")
                        pss[h] = ps
                        for c in range(w0 // 512, 2):
                            a = max(w0, 512 * c)
                            nc.tensor.matmul(
                                ps[:, a:512 * (c + 1)],
                                _r(qkT[qsl, 2 + pair, w0:w0 + 128]),
                                _r(qkT[qsl, pair, a:512 * (c + 1)]),
                                start=True, stop=True)
                    for hh in range(2):
                        nc.tensor.matmul(
                            pss[2 * pair + hh][:, w0:w0 + 128], _r(negtri),
                            _r(ident), start=False, stop=True)
                    for hh in range(2):
                        h = 2 * pair + hh
                        nc.scalar.activation(
                            expTs[h][:, EOFF[j]:EOFF[j] + (S - w0)],
                            pss[h][:, w0:S], AF.Exp, scale=0.125)
                if j >= 2:
                    emit_attv(j - 2)
                if FILLER:
                    for _ in range(FILLER):
                        fps = psA.tile([128, 256], F32, tag="psA")
                        nc.tensor.matmul(fps, _r(warm[:, 0:128]),
                                         _r(warm[:, 0:256]),
                                         start=True, stop=True)
            emit_attv(ST - 2)
            emit_attv(ST - 1)
            oT = bp2.tile([128, ST, ET, 128], MMDT, tag="oT")
            for half in (0, 1):
                emit_transpose_half(o_sb, oT, half)
            return oT

        def emit_p3(b, xs, oT):
            # ---- out_proj + residual -> x2 ----
            x2 = bp1.tile([128, ST, E], F32, tag="x2")
            for t in range(ST):
                ps = psA.tile([128, E], F32, tag="psA")
                nc.tensor.matmul(ps, _r(oT[:, t, 0, :]),
                                 _r(out_w_sb[:, 0, :]), start=True, stop=False)
                nc.tensor.matmul(ps, _r(oT[:, t, 1, :]),
                                 _r(out_w_sb[:, 1, :]), start=False, stop=False)
                nc.tensor.matmul(ps, _r(ones_row[0:1, 128 * t:128 * (t + 1)]),
                                 _r(outb_sb[0:1, :]), start=False, stop=True)
                nc.vector.tensor_add(x2[:, t, :], ps, xs[:, t, :])

            # ---- LN2 -> h2 (bf16) -> h2T via XBAR ----
            h2 = bp2.tile([128, ST, E], MMDT, tag="h1")
            h2T = bp2.tile([128, ST, ET, 128], MMDT, tag="hT")
            rstd = sp.tile([128, ST], F32, tag="rstd")
            nmr = sp.tile([128, ST], F32, tag="nmr")
            for half in (0, 1):
                emit_ln_half(x2, lambda t: h2[:, t, :], half, rstd, nmr)
                emit_transpose_half(h2, h2T, half)

            # ---- MLP fc + relu (uT feature-major) ----
            uT = bp1.tile([128, FT, S], MMDT, tag="uT")
            for m in range(FT):
                ps = psB.tile([128, S], F32, tag="psB")
                for c in range(2):
                    sl = slice(512 * c, 512 * (c + 1))
                    nc.tensor.matmul(ps[:, sl], _r(fc_w_sb[:, 0, 128 * m:128 * (m + 1)]),
                                     _r(h2T[:, 4 * c:4 * c + 4, 0, :]),
                                     start=True, stop=False)
                    nc.tensor.matmul(ps[:, sl], _r(fc_w_sb[:, 1, 128 * m:128 * (m + 1)]),
                                     _r(h2T[:, 4 * c:4 * c + 4, 1, :]),
                                     start=False, stop=True)
                nc.scalar.activation(uT[:, m, :], ps, AF.Relu,
                                     bias=fc_bt_sb[:, m:m + 1])
                if FILLER:
                    fps = psO.tile([128, 512], F32, tag="psO")
                    nc.tensor.matmul(fps, _r(warm[:, 0:128]), _r(warm),
                                     start=True, stop=True)

            # ---- proj + residual -> y (reuses the xs tile in place) ----
            for t in range(ST):
                ps = psA.tile([128, E], F32, tag="psA")
                for m in range(FT):
                    nc.tensor.matmul(ps, _r(uT[:, m, 128 * t:128 * (t + 1)]),
                                     _r(proj_w_sb[:, m, :]),
                                     start=(m == 0), stop=False)
                nc.tensor.matmul(ps, _r(ones_row[0:1, 128 * t:128 * (t + 1)]),
                                 _r(projb_sb[0:1, :]), start=False, stop=True)
                nc.vector.tensor_add(xs[:, t, :], ps, x2[:, t, :])
            for half in (0, 1):
                t0 = 4 * half
                nc.sync.dma_start(
                    out=y_out[b, 128 * t0:128 * (t0 + 4), :].rearrange(
                        "(t p) e -> p t e", p=128),
                    in_=xs[:, t0:t0 + 4, :])

        for _rep in range(reps):
            xs_tiles = []
            for b in range(BPC):
                xs = bp2.tile([128, ST, E], F32, tag="xs", name=f"xs{b}")
                for half in (0, 1):
                    t0 = 4 * half
                    nc.sync.dma_start(
                        out=xs[:, t0:t0 + 4, :],
                        in_=x_in[b, 128 * t0:128 * (t0 + 4), :].rearrange(
                            "(t p) e -> p t e", p=128))
                xs_tiles.append(xs)
            # strict phase order so every engine queue's FIFO matches true
            # dependency order (an out-of-order emission head-blocks the
            # in-order engine queues on cross-batch waits)
            qv0 = emit_p1(0, xs_tiles[0])
            emit_late_weights()
            qv1 = emit_p1(1, xs_tiles[1])
            oT0 = emit_p2(0, *qv0)
            oT1 = emit_p2(1, *qv1)
            emit_p3(0, xs_tiles[0], oT0)
            emit_p3(1, xs_tiles[1], oT1)

        for p in (psO, psB, psA, op, ep, bp1, bp2, sp, wp):
            p.release()

    nc.compile()
    return nc


def host_prep(inputs):
    """Fold LN params into weights; build the DRAM-side weight layouts."""
    f = np.float32
    qkv_w = np.asarray(inputs["qkv_w"], f)
    qkv_b = np.asarray(inputs["qkv_b"], f)
    ln1_g = np.asarray(inputs["ln1_g"], f)
    ln1_b = np.asarray(inputs["ln1_b"], f)
    ln2_g = np.asarray(inputs["ln2_g"], f)
    ln2_b = np.asarray(inputs["ln2_b"], f)
    fc_w = np.asarray(inputs["fc_w"], f)
    fc_b = np.asarray(inputs["fc_b"], f)

    W1 = qkv_w * ln1_g[:, None]
    b1 = qkv_b + ln1_b @ qkv_w
    W2 = fc_w * ln2_g[:, None]
    b2 = fc_b + ln2_b @ fc_w

    qk_w = np.ascontiguousarray(W1[:, :512])
    qk_bc = np.ascontiguousarray(b1[:512].reshape(4, 128).T)
    wv = np.zeros((E, H * (D + 1)), f)
    bv = np.zeros((1, H * (D + 1)), f)
    for h in range(H):
        wv[:, 65 * h:65 * h + 64] = W1[:, 512 + 64 * h:512 + 64 * (h + 1)]
        bv[0, 65 * h:65 * h + 64] = b1[512 + 64 * h:512 + 64 * (h + 1)]
        bv[0, 65 * h + 64] = 1.0
    fc_bt = np.ascontiguousarray(b2.reshape(FT, 128).T)

    import ml_dtypes
    wdt = np.float32 if MM_DTYPE == "f32r" else ml_dtypes.bfloat16

    return {
        "qk_w": qk_w.astype(wdt),
        "qk_bc": qk_bc,
        "wv": wv.astype(wdt),
        "bv_row": bv.astype(wdt),
        "out_w": np.asarray(inputs["out_w"], f).astype(wdt),
        "outb_row": np.asarray(inputs["out_b"], f).reshape(1, E).astype(wdt),
        "fc_w": W2.astype(wdt),
        "fc_bt": fc_bt,
        "proj_w": np.asarray(inputs["proj_w"], f).astype(wdt),
        "projb_row": np.asarray(inputs["proj_b"], f).reshape(1, E).astype(wdt),
    }


_NC_CACHE = None


def _get_nc():
    global _NC_CACHE
    if _NC_CACHE is None:
        _NC_CACHE = build_bass()
    return _NC_CACHE


def run(inputs, trace=False):
    from concourse.bass_utils import run_bass_kernel_spmd

    nc = _get_nc()
    weights = host_prep(inputs)
    x = np.asarray(inputs["x"], np.float32)
    in_maps = []
    for c in range(NCORES):
        m = dict(weights)
        m["x"] = np.ascontiguousarray(x[BPC * c:BPC * (c + 1)])
        in_maps.append(m)
    res = run_bass_kernel_spmd(nc, in_maps, core_ids=list(range(NCORES)),
                               trace=trace)
    y = np.concatenate([res.results[c]["y"] for c in range(NCORES)], axis=0)
    return y, res


def kernel(**inputs):
    y, _ = run(inputs)
    return y


# revision 34
# speedup vs baseline: 1.1127x; 1.1032x over previous
"""Trainium2 Bass kernel for a GPT-style transformer block.

Reference computation (per batch element):
    h  = LN1(x);  qkv = h @ qkv_w + qkv_b
    att = causal_softmax(q @ k.T / sqrt(64));  o = att @ v
    x  = x + o @ out_w + out_b
    h  = LN2(x);  u = relu(h @ fc_w + fc_b)
    y  = x + u @ proj_w + proj_b

Shapes: x [16, 1024, 256], 4 heads x 64, MLP hidden 1024.

Strategy: pure data-parallel over batch, 2 batch elements per core on 8
cores, no collectives.  Within a core:
  - All PE operands are bf16 (fp32 PSUM accumulation).  LN gamma/beta are
    folded into the following matmul weights on the host.
  - Seq-major <-> feature-major layout changes run on the DMA engines'
    XBAR transpose (SBUF->SBUF, 2-byte dtype), freeing the PE and ACT
    engines entirely; transposed tensors use the XBAR's native
    [dim_lo(128), t, et, q(128)] layout.
  - Attention scores are computed transposed (scoresT[k, q]); the causal
    mask is applied by accumulating a -1e9 strict-triangular matrix onto
    the diagonal PSUM block with one extra matmul (exp underflows to 0).
  - att@V runs in [q, dims] orientation (expT chunks are the stationary
    operand), so softmax denominators (from an all-ones column appended
    to V) land on the same partition as their row: normalization is a
    per-partition scalar multiply; reciprocals via the fast approximate
    DVE op.  The normalized o is XBAR-transposed back to feature-major
    for out_proj.
  - Engine balance: exp/relu/copies on ACT, stats/evac/recip/residuals
    on DVE, LN apply on GpSimd, masks folded into PE matmuls.
  - Input x and LN/transpose flow in half-batches of 4 seq tiles so the
    first qkv matmuls start as early as possible; a short run of dummy
    matmuls at kernel start keeps the PE busy while DMAs land, so the
    HAM clock gate is open when real work arrives.
"""

import sys

sys.path.insert(0, "/opt/trn_rl_repo")

import numpy as np

import concourse.bass as bass
import concourse.bacc as bacc
import concourse.tile as tile
from concourse import mybir
from concourse.masks import make_identity, make_upper_triangular

# Restrict the activation-table chooser to the one set that contains every
# function this kernel uses (exp, ln, copy/identity, relu) so the ACT
# engine never thrashes table loads.
_orig_get_act_tables = bacc.get_activation_tables


def _one_set_tables(module_arch):
    tabs = _orig_get_act_tables(module_arch)
    return {name: (fns if name == "natural_log_exp_and_others" else set())
            for name, fns in tabs.items()}


bacc.get_activation_tables = _one_set_tables

F32 = mybir.dt.float32
F32R = mybir.dt.float32r
BF16 = mybir.dt.bfloat16
AF = mybir.ActivationFunctionType
ALU = mybir.AluOpType

import os as _os

MM_DTYPE = _os.environ.get("BASS_MM_DTYPE", "bf16")
WARMUP_MM = int(_os.environ.get("BASS_WARMUP_MM", "28"))
# dummy matmuls per attention j-step that keep the PE activity monitor from
# re-throttling the clock during the ACT-bound exp stretches
FILLER = int(_os.environ.get("BASS_FILLER", "4"))


def _mmdt():
    return F32R if MM_DTYPE == "f32r" else BF16

NCORES = 8
B = 16
BPC = B // NCORES  # 2 batch elements per core
S = 1024
E = 256
H = 4
D = 64
FF = 1024
ST = S // 128  # 8 seq tiles
ET = E // 128  # 2 feature tiles
FT = FF // 128  # 8 mlp-hidden tiles
EPS = 1e-5

# ragged offsets for the causal expT store: tile j holds q in [128j, S)
EOFF = [0]
for _j in range(ST):
    EOFF.append(EOFF[-1] + (S - 128 * _j))
ETOT = EOFF[ST]  # 4608


def _r(ap):
    """View an fp32 AP as float32r (no-op for bf16/f32r tiles)."""
    if ap.dtype in (F32R, BF16):
        return ap
    return ap.bitcast(F32R)


def _bcast(ap_1d, parts):
    """Broadcast a 1-partition AP across `parts` partitions (step-0 AP)."""
    return bass.AP(tensor=ap_1d.tensor, offset=ap_1d.offset,
                   ap=[[0, parts]] + list(ap_1d.ap))


def build_bass(reps=1):
    MMDT = _mmdt()
    nc = bacc.Bacc(None, target_bir_lowering=False, debug=False)

    # ---- DRAM I/O ----
    x_in = nc.dram_tensor("x", [BPC, S, E], F32, kind="ExternalInput")
    qk_w = nc.dram_tensor("qk_w", [E, 512], MMDT, kind="ExternalInput")
    qk_bc = nc.dram_tensor("qk_bc", [128, 4], F32, kind="ExternalInput")
    wv = nc.dram_tensor("wv", [E, H * (D + 1)], MMDT, kind="ExternalInput")
    bv_row = nc.dram_tensor("bv_row", [1, H * (D + 1)], MMDT, kind="ExternalInput")
    out_w = nc.dram_tensor("out_w", [E, E], MMDT, kind="ExternalInput")
    outb_row = nc.dram_tensor("outb_row", [1, E], MMDT, kind="ExternalInput")
    fc_w = nc.dram_tensor("fc_w", [E, FF], MMDT, kind="ExternalInput")
    fc_bt = nc.dram_tensor("fc_bt", [128, FT], F32, kind="ExternalInput")
    proj_w = nc.dram_tensor("proj_w", [FF, E], MMDT, kind="ExternalInput")
    projb_row = nc.dram_tensor("projb_row", [1, E], MMDT, kind="ExternalInput")
    y_out = nc.dram_tensor("y", [BPC, S, E], F32, kind="ExternalOutput")
    # scratch for the softmax-denominator reshape round trip
    sums_hbm = nc.dram_tensor("sums_hbm", [BPC, H, S], F32, kind="Internal")
    rec_hbm = nc.dram_tensor("rec_hbm", [BPC, H, S], F32, kind="Internal")

    VA = H * (D + 1)  # 260

    with tile.TileContext(nc) as tc:
        wp = tc.alloc_tile_pool(name="weights", bufs=1)
        sp = tc.alloc_tile_pool(name="small", bufs=2)
        bp2 = tc.alloc_tile_pool(name="big2", bufs=2)
        bp1 = tc.alloc_tile_pool(name="big1", bufs=1)
        ep = tc.alloc_tile_pool(name="expt", bufs=1)
        op = tc.alloc_tile_pool(name="oscp", bufs=4)
        psA = tc.alloc_tile_pool(name="psA", bufs=2, space="PSUM")
        psB = tc.alloc_tile_pool(name="psB", bufs=2, space="PSUM")
        psO = tc.alloc_tile_pool(name="psO", bufs=2, space="PSUM")

        # ---- constants first: the gpsimd queue must build the masks before
        # anything else so the first transposes/scores aren't blocked ----
        eps_col = wp.tile([128, 1], F32)
        nc.vector.memset(eps_col, EPS)
        ones_row = wp.tile([1, S], MMDT)
        nc.vector.memset(ones_row, 1.0)
        ident = wp.tile([128, 128], MMDT)
        make_identity(nc, ident)
        # additive causal mask for the scoresT diagonal block:
        # matmul(negtri.T @ ident) adds -1e9 where q < k
        negtri = wp.tile([128, 128], MMDT)
        make_upper_triangular(nc, negtri, val=-1e9, diag=False)

        # ---- PE warmup: keep the HAM clock gate open during input DMA ----
        warm = wp.tile([128, 512], MMDT)
        nc.vector.memset(warm, 0.25)
        if WARMUP_MM:
            for _ in range(WARMUP_MM):
                wps = psO.tile([128, 512], F32, tag="psO")
                nc.tensor.matmul(wps, _r(warm[:, 0:128]), _r(warm),
                                 start=True, stop=True)

        # ---- weights needed early (attention input projections) ----
        qk_w_sb = wp.tile([128, ET, 512], MMDT)
        nc.gpsimd.dma_start(out=qk_w_sb, in_=qk_w[:, :].rearrange("(t p) c -> p t c", p=128))
        qk_bc_sb = wp.tile([128, 4], F32)
        nc.gpsimd.dma_start(out=qk_bc_sb, in_=qk_bc[:, :])
        wv_sb = wp.tile([128, ET, VA], MMDT)
        nc.gpsimd.dma_start(out=wv_sb, in_=wv[:, :].rearrange("(t p) c -> p t c", p=128))
        bv_sb = wp.tile([1, VA], MMDT)
        nc.gpsimd.dma_start(out=bv_sb, in_=bv_row[:, :])

        # late weights: tiles allocated now, DMAs emitted after p1(0) so the
        # gpsimd queue reaches the first LN applies quickly
        out_w_sb = wp.tile([128, ET, E], MMDT)
        outb_sb = wp.tile([1, E], MMDT)
        fc_w_sb = wp.tile([128, ET, FF], MMDT)
        fc_bt_sb = wp.tile([128, FT], F32)
        proj_w_sb = wp.tile([128, FT, E], MMDT)
        projb_sb = wp.tile([1, E], MMDT)

        def emit_late_weights():
            nc.gpsimd.dma_start(out=out_w_sb, in_=out_w[:, :].rearrange("(t p) c -> p t c", p=128))
            nc.gpsimd.dma_start(out=outb_sb, in_=outb_row[:, :])
            nc.gpsimd.dma_start(out=fc_w_sb, in_=fc_w[:, :].rearrange("(t p) c -> p t c", p=128))
            nc.gpsimd.dma_start(out=fc_bt_sb, in_=fc_bt[:, :])
            nc.gpsimd.dma_start(out=proj_w_sb, in_=proj_w[:, :].rearrange("(t p) c -> p t c", p=128))
            nc.gpsimd.dma_start(out=projb_sb, in_=projb_row[:, :])

        def emit_ln_half(src, dst_fn, half, rstd, nmr):
            """LayerNorm of seq tiles [4*half, 4*half+4): stats on DVE, rstd
            on ACT, apply on GpSimd -> bf16 dst."""
            t0 = 4 * half
            stats = sp.tile([128, 4, 6], F32, tag="bnstats")
            mv = sp.tile([128, 4, 2], F32, tag="bnaggr")
            for i in range(4):
                nc.vector.bn_stats(out=stats[:, i, :], in_=src[:, t0 + i, :])
                nc.vector.bn_aggr(out=mv[:, i, :], in_=stats[:, i, :])
            sl = slice(t0, t0 + 4)
            # rstd = exp(-0.5 * ln(var + eps))
            nc.scalar.activation(rstd[:, sl], mv[:, :, 1], AF.Ln, bias=eps_col)
            nc.scalar.activation(rstd[:, sl], rstd[:, sl], AF.Exp, scale=-0.5)
            nc.vector.tensor_mul(nmr[:, sl], mv[:, :, 0], rstd[:, sl])
            nc.vector.tensor_scalar_mul(nmr[:, sl], nmr[:, sl], -1.0)
            for i in range(4):
                t = t0 + i
                nc.gpsimd.tensor_scalar(
                    out=dst_fn(t), in0=src[:, t, :],
                    scalar1=rstd[:, t:t + 1], scalar2=nmr[:, t:t + 1],
                    op0=ALU.mult, op1=ALU.add)

        def emit_transpose_half(h_sb, hT, half):
            """XBAR transpose seq tiles [4h, 4h+4) of h_sb [128, ST, E] into
            hT [128, ST, ET, 128] (layout [e_lo, t, et, q])."""
            t0 = 4 * half
            nc.sync.dma_start_transpose(
                out=hT[:, t0:t0 + 4, :, :], in_=h_sb[:, t0:t0 + 4, :])

        def emit_p1(b, xs):
            # ---- LN1 -> h1 (bf16) -> h1T via XBAR, in halves ----
            h1 = bp2.tile([128, ST, E], MMDT, tag="h1")
            h1T = bp2.tile([128, ST, ET, 128], MMDT, tag="hT")
            rstd = sp.tile([128, ST], F32, tag="rstd")
            nmr = sp.tile([128, ST], F32, tag="nmr")
            for half in (0, 1):
                emit_ln_half(xs, lambda t: h1[:, t, :], half, rstd, nmr)
                emit_transpose_half(h1, h1T, half)

            # ---- qkv ----
            # q/k feature-major: qkT[:, m, :]; m=0: q heads 0-1, m=1: q heads
            # 2-3, m=2: k heads 0-1, m=3: k heads 2-3 (rows = 2x64 head dims)
            qkT = bp2.tile([128, 4, S], MMDT, tag="qkT")
            for m in range(4):
                ps = psB.tile([128, S], F32, tag="psB")
                for c in range(2):
                    sl = slice(512 * c, 512 * (c + 1))
                    nc.tensor.matmul(ps[:, sl], _r(qk_w_sb[:, 0, 128 * m:128 * (m + 1)]),
                                     _r(h1T[:, 4 * c:4 * c + 4, 0, :]),
                                     start=True, stop=False)
                    nc.tensor.matmul(ps[:, sl], _r(qk_w_sb[:, 1, 128 * m:128 * (m + 1)]),
                                     _r(h1T[:, 4 * c:4 * c + 4, 1, :]),
                                     start=False, stop=True)
                    # evacuate + per-partition bias add per chunk
                    nc.vector.tensor_scalar(
                        out=qkT[:, m, sl], in0=ps[:, sl],
                        scalar1=qk_bc_sb[:, m:m + 1],
                        scalar2=None, op0=ALU.add)

            # v seq-major with per-head all-ones column: vv[:, t, 65h+64] == 1
            vv = bp2.tile([128, ST, VA], MMDT, tag="vv")
            for t in range(ST):
                ps = psA.tile([128, VA], F32, tag="psA")
                nc.tensor.matmul(ps, _r(h1T[:, t, 0, :]),
                                 _r(wv_sb[:, 0, :]), start=True, stop=False)
                nc.tensor.matmul(ps, _r(h1T[:, t, 1, :]),
                                 _r(wv_sb[:, 1, :]), start=False, stop=False)
                # bias row (rank-1: ones_col @ bv_row); also writes the 1.0 col
                nc.tensor.matmul(ps, _r(ones_row[0:1, 128 * t:128 * (t + 1)]),
                                 _r(bv_sb[0:1, :]), start=False, stop=True)
                nc.scalar.copy(vv[:, t, :], ps)

            return qkT, vv

        def emit_p2(b, qkT, vv):
            # ---- attention: scores j-loop with att@V zippered in ----
            expTs = {h: ep.tile([128, ETOT], MMDT, tag=f"expT{h}",
                                name=f"expT{h}b{b}") for h in range(H)}
            o_sb = bp2.tile([128, ST, E], MMDT, tag="osb")

            def emit_attv(t):
                po = psO.tile([128, VA], F32, tag="psO")
                for h in range(H):
                    expT = expTs[h]
                    for j in range(t + 1):
                        a = EOFF[j] + 128 * (t - j)
                        nc.tensor.matmul(
                            po[:, 65 * h:65 * h + 65],
                            _r(expT[:, a:a + 128]),
                            _r(vv[:, j, 65 * h:65 * h + 65]),
                            start=(j == 0), stop=(j == t))
                pr = po.rearrange("p (h x) -> p h x", x=65)
                sums = sp.tile([128, H, 1], F32, tag="sums")
                nc.vector.tensor_copy(sums, pr[:, :, 64:65])
                rec4 = sp.tile([128, H, 1], F32, tag="rec4")
                nc.vector.reciprocal_approx_fast(rec4, sums)
                for h in range(H):
                    nc.vector.tensor_scalar(
                        out=o_sb[:, t, 64 * h:64 * h + 64], in0=pr[:, h, 0:64],
                        scalar1=rec4[:, h, :], scalar2=None, op0=ALU.mult)

            for j in range(ST):
                w0 = 128 * j
                pss = {}
                for pair in range(2):
                    for hh in range(2):
                        h = 2 * pair + hh
                        qsl = slice(64 * hh, 64 * hh + 64)
                        ps = psB.tile([128, S], F32, tag="psB", name=f"psSC{h}")
                        pss[h] = ps
                        for c in range(w0 // 512, 2):
                            a = max(w0, 512 * c)
                            nc.tensor.matmul(
                                ps[:, a:512 * (c + 1)],
                                _r(qkT[qsl, 2 + pair, w0:w0 + 128]),
                                _r(qkT[qsl, pair, a:512 * (c + 1)]),
                                start=True, stop=True)
                    for hh in range(2):
                        nc.tensor.matmul(
                            pss[2 * pair + hh][:, w0:w0 + 128], _r(negtri),
                            _r(ident), start=False, stop=True)
                    for hh in range(2):
                        h = 2 * pair + hh
                        nc.scalar.activation(
                            expTs[h][:, EOFF[j]:EOFF[j] + (S - w0)],
                            pss[h][:, w0:S], AF.Exp, scale=0.125)
                if j >= 2:
                    emit_attv(j - 2)
                if FILLER:
                    for _ in range(FILLER):
                        fps = psA.tile([128, 256], F32, tag="psA")
                        nc.tensor.matmul(fps, _r(warm[:, 0:128]),
                                         _r(warm[:, 0:256]),
                                         start=True, stop=True)
            emit_attv(ST - 2)
            emit_attv(ST - 1)
            oT = bp2.tile([128, ST, ET, 128], MMDT, tag="oT")
            for half in (0, 1):
                emit_transpose_half(o_sb, oT, half)
            return oT

        def emit_p3a(b, xs, oT):
            # ---- out_proj + residual -> x2; LN2; fc + relu -> uT ----
            x2 = bp2.tile([128, ST, E], F32, tag="x2")
            for t in range(ST):
                ps = psA.tile([128, E], F32, tag="psA")
                nc.tensor.matmul(ps, _r(oT[:, t, 0, :]),
                                 _r(out_w_sb[:, 0, :]), start=True, stop=False)
                nc.tensor.matmul(ps, _r(oT[:, t, 1, :]),
                                 _r(out_w_sb[:, 1, :]), start=False, stop=False)
                nc.tensor.matmul(ps, _r(ones_row[0:1, 128 * t:128 * (t + 1)]),
                                 _r(outb_sb[0:1, :]), start=False, stop=True)
                nc.vector.tensor_add(x2[:, t, :], ps, xs[:, t, :])

            h2 = bp2.tile([128, ST, E], MMDT, tag="h1")
            h2T = bp2.tile([128, ST, ET, 128], MMDT, tag="hT")
            rstd = sp.tile([128, ST], F32, tag="rstd")
            nmr = sp.tile([128, ST], F32, tag="nmr")
            for half in (0, 1):
                emit_ln_half(x2, lambda t: h2[:, t, :], half, rstd, nmr)
                emit_transpose_half(h2, h2T, half)

            uT = bp2.tile([128, FT, S], MMDT, tag="uT")
            for m in range(FT):
                ps = psB.tile([128, S], F32, tag="psB")
                for c in range(2):
                    sl = slice(512 * c, 512 * (c + 1))
                    nc.tensor.matmul(ps[:, sl], _r(fc_w_sb[:, 0, 128 * m:128 * (m + 1)]),
                                     _r(h2T[:, 4 * c:4 * c + 4, 0, :]),
                                     start=True, stop=False)
                    nc.tensor.matmul(ps[:, sl], _r(fc_w_sb[:, 1, 128 * m:128 * (m + 1)]),
                                     _r(h2T[:, 4 * c:4 * c + 4, 1, :]),
                                     start=False, stop=True)
                nc.scalar.activation(uT[:, m, :], ps, AF.Relu,
                                     bias=fc_bt_sb[:, m:m + 1])
                if FILLER:
                    fps = psO.tile([128, 512], F32, tag="psO")
                    nc.tensor.matmul(fps, _r(warm[:, 0:128]), _r(warm),
                                     start=True, stop=True)
            return x2, uT

        def emit_p3b(b, xs, x2, uT):
            # ---- proj + residual -> y (reuses the xs tile in place) ----
            for t in range(ST):
                ps = psA.tile([128, E], F32, tag="psA")
                for m in range(FT):
                    nc.tensor.matmul(ps, _r(uT[:, m, 128 * t:128 * (t + 1)]),
                                     _r(proj_w_sb[:, m, :]),
                                     start=(m == 0), stop=False)
                nc.tensor.matmul(ps, _r(ones_row[0:1, 128 * t:128 * (t + 1)]),
                                 _r(projb_sb[0:1, :]), start=False, stop=True)
                nc.vector.tensor_add(xs[:, t, :], ps, x2[:, t, :])
            for half in (0, 1):
                t0 = 4 * half
                nc.sync.dma_start(
                    out=y_out[b, 128 * t0:128 * (t0 + 4), :].rearrange(
                        "(t p) e -> p t e", p=128),
                    in_=xs[:, t0:t0 + 4, :])

        for _rep in range(reps):
            xs_tiles = []
            for b in range(BPC):
                xs = bp2.tile([128, ST, E], F32, tag="xs", name=f"xs{b}")
                for half in (0, 1):
                    t0 = 4 * half
                    nc.sync.dma_start(
                        out=xs[:, t0:t0 + 4, :],
                        in_=x_in[b, 128 * t0:128 * (t0 + 4), :].rearrange(
                            "(t p) e -> p t e", p=128))
                xs_tiles.append(xs)
            # strict phase order so every engine queue's FIFO matches true
            # dependency order (an out-of-order emission head-blocks the
            # in-order engine queues on cross-batch waits)
            qv0 = emit_p1(0, xs_tiles[0])
            emit_late_weights()
            qv1 = emit_p1(1, xs_tiles[1])
            oT0 = emit_p2(0, *qv0)
            oT1 = emit_p2(1, *qv1)
            # p3 split: batch 1's out_proj/fc fill batch 0's relu-wait and
            # vice versa for the proj tails
            xu0 = emit_p3a(0, xs_tiles[0], oT0)
            xu1 = emit_p3a(1, xs_tiles[1], oT1)
            emit_p3b(0, xs_tiles[0], *xu0)
            emit_p3b(1, xs_tiles[1], *xu1)

        for p in (psO, psB, psA, op, ep, bp1, bp2, sp, wp):
            p.release()

    nc.compile()
    return nc


def host_prep(inputs):
    """Fold LN params into weights; build the DRAM-side weight layouts."""
    f = np.float32
    qkv_w = np.asarray(inputs["qkv_w"], f)
    qkv_b = np.asarray(inputs["qkv_b"], f)
    ln1_g = np.asarray(inputs["ln1_g"], f)
    ln1_b = np.asarray(inputs["ln1_b"], f)
    ln2_g = np.asarray(inputs["ln2_g"], f)
    ln2_b = np.asarray(inputs["ln2_b"], f)
    fc_w = np.asarray(inputs["fc_w"], f)
    fc_b = np.asarray(inputs["fc_b"], f)

    W1 = qkv_w * ln1_g[:, None]
    b1 = qkv_b + ln1_b @ qkv_w
    W2 = fc_w * ln2_g[:, None]
    b2 = fc_b + ln2_b @ fc_w

    qk_w = np.ascontiguousarray(W1[:, :512])
    qk_bc = np.ascontiguousarray(b1[:512].reshape(4, 128).T)
    wv = np.zeros((E, H * (D + 1)), f)
    bv = np.zeros((1, H * (D + 1)), f)
    for h in range(H):
        wv[:, 65 * h:65 * h + 64] = W1[:, 512 + 64 * h:512 + 64 * (h + 1)]
        bv[0, 65 * h:65 * h + 64] = b1[512 + 64 * h:512 + 64 * (h + 1)]
        bv[0, 65 * h + 64] = 1.0
    fc_bt = np.ascontiguousarray(b2.reshape(FT, 128).T)

    import ml_dtypes
    wdt = np.float32 if MM_DTYPE == "f32r" else ml_dtypes.bfloat16

    return {
        "qk_w": qk_w.astype(wdt),
        "qk_bc": qk_bc,
        "wv": wv.astype(wdt),
        "bv_row": bv.astype(wdt),
        "out_w": np.asarray(inputs["out_w"], f).astype(wdt),
        "outb_row": np.asarray(inputs["out_b"], f).reshape(1, E).astype(wdt),
        "fc_w": W2.astype(wdt),
        "fc_bt": fc_bt,
        "proj_w": np.asarray(inputs["proj_w"], f).astype(wdt),
        "projb_row": np.asarray(inputs["proj_b"], f).reshape(1, E).astype(wdt),
    }


_NC_CACHE = None


def _get_nc():
    global _NC_CACHE
    if _NC_CACHE is None:
        _NC_CACHE = build_bass()
    return _NC_CACHE


def run(inputs, trace=False):
    from concourse.bass_utils import run_bass_kernel_spmd

    nc = _get_nc()
    weights = host_prep(inputs)
    x = np.asarray(inputs["x"], np.float32)
    in_maps = []
    for c in range(NCORES):
        m = dict(weights)
        m["x"] = np.ascontiguousarray(x[BPC * c:BPC * (c + 1)])
        in_maps.append(m)
    res = run_bass_kernel_spmd(nc, in_maps, core_ids=list(range(NCORES)),
                               trace=trace)
    y = np.concatenate([res.results[c]["y"] for c in range(NCORES)], axis=0)
    return y, res


def kernel(**inputs):
    y, _ = run(inputs)
    return y
